# revision 6
# baseline (speedup 1.0000x reference)
"""Trainium2 Bass kernel for nn_CrossAttention (B=4, H=8, D=64, C=512, N=M=2048).

Sharding: 8 cores = batch (4) x head-group (2). Core c handles batch b=c//2
and heads hg*4..hg*4+4 with hg=c%2 (tensor parallel on inner_dim). Each core
emits a full-shape partial y (its Wo column block times its heads' attention
output, plus bias on hg==0); the host unshard sums the two partials per batch.

Per-core math (all on-device):
  q  = Wq[hg] @ x             [256, 2048]
  k  = Wk[hg] @ ctx           [256, 2048]
  vT = ctx.T @ Wv[hg].T       [2048, 256]   (built into the ones-augmented
                                             per-j-chunk layout for PV)
  per local head h: simT[j,i] = sum_d k[d,j] q[d,i];  p = exp(simT/8)
  out_aug = [vT_h | 1].T @ p  [65, 2048]    (row 64 = softmax denominator)
  out_h   = out_aug[:64] / out_aug[64]
  y_part = Wo[:, hg cols] @ out (+ bo)      [512, 2048]

All matmul operands are bf16 (fp32 PSUM accumulation): LDWEIGHTS gets the
fast (FWL) path and input DMA halves. The two sim matmuls per j-chunk use
K=64 row groups at base partitions 0/64 and are emitted back-to-back so they
run concurrently in disjoint PE row groups. Softmax skips max-subtraction
(|sim/8| small for this distribution); the denominator rides the PV matmul
as a ones column. Division happens off the PE: DVE drain, approx reciprocal,
gpsimd partition-broadcast, multiply. The y projection accumulates both
inner-chunk matmuls in PSUM (one DVE bias-add readout per tile) and is
drained pass-by-pass so only the last i-tile's y work trails the final pass.
"""

from collections import deque
from contextlib import ExitStack

import numpy as np
import ml_dtypes

import concourse.bass as bass
import concourse.mybir as mybir
import concourse.tile as tile
from concourse import bacc
from concourse.bass_utils import run_bass_kernel_spmd

FP = mybir.dt.float32
BF16 = mybir.dt.bfloat16
EXP = mybir.ActivationFunctionType.Exp
NP_BF16 = ml_dtypes.bfloat16

P = 128
H, D = 8, 64
C = 512             # query_dim == full inner_dim
N, M = 2048, 2048
HL = 4              # local heads per core
HPL = 2             # local head pairs
CIN = HL * D        # local inner dim = 256
CC = C // P         # 4 contraction chunks for q/k/v projections
IT = N // 512       # 4 query i-tiles
JC = M // P         # 16 context chunks
NT = M // 512       # 4 context column blocks
ICY = CIN // P      # 2 inner chunks for the y projection
SCALE = float(D) ** -0.5
N_CORES = 8
NWARM = 24


def _build_program():
    nc = bacc.Bacc("TRN2")
    x = nc.dram_tensor("x", [P, IT * CC * 512], BF16, kind="ExternalInput")
    ctx = nc.dram_tensor("ctx", [P, NT * CC * 512], BF16, kind="ExternalInput")
    wq = nc.dram_tensor("wq", [P, CC * CIN], BF16, kind="ExternalInput")
    wk = nc.dram_tensor("wk", [P, CC * CIN], BF16, kind="ExternalInput")
    wv = nc.dram_tensor("wv", [P, CC * CIN], BF16, kind="ExternalInput")
    wo = nc.dram_tensor("wo", [P, ICY * 512], BF16, kind="ExternalInput")
    bo = nc.dram_tensor("bo", [P, CC], FP, kind="ExternalInput")
    y = nc.dram_tensor("y", [P, CC * N], BF16, kind="ExternalOutput")

    with tile.TileContext(nc) as tc:
        _emit(tc, x, ctx, wq, wk, wv, wo, bo, y)
    nc.finalize()
    return nc


def _emit(tc, x, ctx, wq, wk, wv, wo, bo, y):
    nc = tc.nc
    with ExitStack() as st:
        wpool = st.enter_context(tc.tile_pool(name="weights", bufs=1))
        apool = st.enter_context(tc.tile_pool(name="acts", bufs=1))
        ppool = st.enter_context(tc.tile_pool(name="pexp", bufs=3))
        spool = st.enter_context(tc.tile_pool(name="small", bufs=2))
        psim = st.enter_context(tc.tile_pool(name="psim", bufs=2, space="PSUM"))
        ppv = st.enter_context(tc.tile_pool(name="ppv", bufs=2, space="PSUM"))
        pmisc = st.enter_context(tc.tile_pool(name="pmisc", bufs=2, space="PSUM"))

        # ---- input loads, ordered so pass(0,0) starts ASAP ----
        # pass(0,0) j=0 needs only wk+ctx0 (k chunk 0), wv (inline v), x0+wq
        # (q tile 0); later ctx/x blocks stream in under the pass.
        wq_s = wpool.tile([P, CC * CIN], BF16, tag="wq")
        nc.sync.dma_start(out=wq_s, in_=wq[:, :])
        wk_s = wpool.tile([P, CC * CIN], BF16, tag="wk")
        nc.sync.dma_start(out=wk_s, in_=wk[:, :])
        ctx_s = apool.tile([P, NT * CC * 512], BF16, tag="ctx")
        nc.sync.dma_start(out=ctx_s[:, 0:2048], in_=ctx[:, 0:2048])
        wv_s = wpool.tile([P, CC * CIN], BF16, tag="wv")
        nc.sync.dma_start(out=wv_s, in_=wv[:, :])
        x_s = apool.tile([P, IT * CC * 512], BF16, tag="x")
        nc.sync.dma_start(out=x_s[:, 0:2048], in_=x[:, 0:2048])
        for nb in range(1, NT):
            nc.sync.dma_start(
                out=ctx_s[:, nb * 2048:(nb + 1) * 2048],
                in_=ctx[:, nb * 2048:(nb + 1) * 2048],
            )
        for it in range(1, IT):
            nc.sync.dma_start(
                out=x_s[:, it * 2048:(it + 1) * 2048],
                in_=x[:, it * 2048:(it + 1) * 2048],
            )
        wo_s = wpool.tile([P, ICY * 512], BF16, tag="wo")
        nc.sync.dma_start(out=wo_s, in_=wo[:, :])
        bo_s = wpool.tile([P, CC], FP, tag="bo")
        nc.sync.dma_start(out=bo_s, in_=bo[:, :])

        # ---- persistent SBUF intermediates ----
        # q/k: local head pair hp at cols hp*2048 + it(or nt)*512 + n
        q_s = apool.tile([P, HPL * N], BF16, tag="q")
        k_s = apool.tile([P, HPL * M], BF16, tag="k")
        # v aug: j-chunk j at cols j*(HL*65), local head h at sub-cols h*65
        vaug = apool.tile([P, JC * (HL * 65)], BF16, tag="vaug")
        # attention out, local inner chunk ic at cols ic*2048 + it*512
        out_s = apool.tile([P, ICY * N], BF16, tag="out")
        # full-shape partial y staging (DMA'd out per tile)
        y_acc = apool.tile([P, CC * N], BF16, tag="yacc")
        # fp32 ones staging for vaug ones columns
        ones_s = wpool.tile([P, P], FP, tag="ones")
        nc.vector.memset(ones_s, 1.0)
        vaug4 = vaug.rearrange("p (j h e) -> p j h e", j=JC, h=HL)
        ones4 = ones_s[:, 0:JC * HL].rearrange("p (j h e) -> p j h e", j=JC, h=HL)
        nc.vector.tensor_copy(out=vaug4[:, :, :, 64:65], in_=ones4)

        # HAM warmup: burn matmuls on the ones tile during the initial DMA
        # wait so the first projections run at the full 2.4GHz clock.
        warm = pmisc.tile([P, 512], FP, tag="scratch", name="warm")
        for w in range(NWARM):
            nc.tensor.matmul(warm[:, 0:P], lhsT=ones_s[:, 0:P],
                             rhs=ones_s[:, 0:P],
                             start=(w == 0), stop=(w == NWARM - 1))
        warm_sink = spool.tile([P, P], FP, tag="warmsink", bufs=1)
        nc.vector.tensor_copy(out=warm_sink, in_=warm[:, 0:P])

        def proj_qk(dst, w_s, oc, rhs_of_cc):
            """One [128, 512] q/k projection tile (local head pair oc)."""
            pt = pmisc.tile([P, 512], FP, tag="scratch")
            for cc in range(CC):
                nc.tensor.matmul(
                    pt,
                    lhsT=w_s[:, cc * CIN + oc * P: cc * CIN + (oc + 1) * P],
                    rhs=rhs_of_cc(cc),
                    start=(cc == 0), stop=(cc == CC - 1),
                )
            nc.vector.tensor_copy(out=dst, in_=pt)

        def emit_q(oc, it):
            proj_qk(q_s[:, oc * N + it * 512: oc * N + (it + 1) * 512], wq_s, oc,
                    lambda cc: x_s[:, it * 2048 + cc * 512: it * 2048 + (cc + 1) * 512])

        def emit_k(oc, nt):
            proj_qk(k_s[:, oc * M + nt * 512: oc * M + (nt + 1) * 512], wk_s, oc,
                    lambda cc: ctx_s[:, nt * 2048 + cc * 512: nt * 2048 + (cc + 1) * 512])

        def emit_v(j):
            nb, jm = j // 4, j % 4
            pt = pmisc.tile([P, 512], FP, tag="scratch")
            for cc in range(CC):
                nc.tensor.matmul(
                    pt[:, 0:CIN],
                    lhsT=ctx_s[:, nb * 2048 + cc * 512 + jm * P:
                               nb * 2048 + cc * 512 + (jm + 1) * P],
                    rhs=wv_s[:, cc * CIN:(cc + 1) * CIN],
                    start=(cc == 0), stop=(cc == CC - 1),
                )
            nc.vector.tensor_copy(
                out=vaug4[:, j, :, 0:64],
                in_=pt[:, 0:CIN].rearrange("p (h e) -> p h e", h=HL),
            )

        def emit_y(oc, nt2):
            """Full y output tile (oc, nt2): both inner chunks accumulate in
            PSUM, then one bias-add readout + DMA."""
            pt = pmisc.tile([P, 512], FP, tag="scratch")
            for ic in range(ICY):
                nc.tensor.matmul(
                    pt,
                    lhsT=wo_s[:, ic * 512 + oc * P: ic * 512 + (oc + 1) * P],
                    rhs=out_s[:, ic * N + nt2 * 512: ic * N + (nt2 + 1) * 512],
                    start=(ic == 0), stop=(ic == ICY - 1),
                )
            ysl = y_acc[:, oc * N + nt2 * 512: oc * N + (nt2 + 1) * 512]
            nc.vector.tensor_scalar_add(out=ysl, in0=pt,
                                        scalar1=bo_s[:, oc:oc + 1])
            nc.sync.dma_start(
                out=y[:, oc * N + nt2 * 512: oc * N + (nt2 + 1) * 512],
                in_=ysl)

        # pinned[i]: projection tiles that MUST be emitted during pass i
        # in its drain slots (j = 1, 4, 7, 10, 13), sized so each tile
        # lands before its first consumer; free: y tiles drained
        # opportunistically.
        pinned = {i: deque() for i in range(HPL * IT)}
        # k(0, nt) is consumed from j = 4*nt of pass 0 onward; slots 1/4/7
        # precede j = 8/12 comfortably (k(0,1) emitted at j=1 finishes ~j=3).
        pinned[0].extend([
            lambda: emit_k(0, 1), lambda: emit_k(0, 2), lambda: emit_k(0, 3),
            lambda: emit_q(0, 1)])
        pinned[1].append(lambda: emit_q(0, 2))
        pinned[2].extend([
            lambda: emit_q(0, 3), lambda: emit_k(1, 0), lambda: emit_k(1, 1)])
        pinned[3].extend([
            lambda: emit_k(1, 2), lambda: emit_k(1, 3), lambda: emit_q(1, 0)])
        for it in range(1, IT):
            pinned[3 + it].append(lambda it=it: emit_q(1, it))
        free = deque()

        # upfront: just enough for pass(0,0) j=0 — q tile 0 and k chunk 0
        emit_q(0, 0)
        emit_k(0, 0)

        def attention_pass(hp, it, emit_v_inline, mine):
            hA, hB = 2 * hp, 2 * hp + 1
            pvA = ppv.tile([65, 512], FP, tag="pv")
            pvB = ppv.tile([65, 512], FP, tag="pv")
            qA = q_s[0:64, hp * N + it * 512: hp * N + (it + 1) * 512]
            qB = q_s[64:128, hp * N + it * 512: hp * N + (it + 1) * 512]
            pts = [None] * JC

            def emit_sim(j):
                # Both K=64 halves back-to-back: base partitions 0/64 give
                # disjoint PE row groups, so the two matmuls run concurrently.
                if emit_v_inline:
                    emit_v(j)
                st_t = psim.tile([P, 1024], FP, tag="sim", name="st_t")
                pt = ppool.tile([P, 1024], BF16, tag="p", name="pt")
                pts[j] = (st_t, pt)
                nc.tensor.matmul(
                    st_t[:, 0:512],
                    lhsT=k_s[0:64, hp * M + j * P: hp * M + (j + 1) * P],
                    rhs=qA,
                )
                nc.tensor.matmul(
                    st_t[:, 512:1024],
                    lhsT=k_s[64:128, hp * M + j * P: hp * M + (j + 1) * P],
                    rhs=qB,
                )
                nc.scalar.activation(out=pt, in_=st_t, func=EXP, scale=SCALE)

            def emit_pv(j):
                pt = pts[j][1]
                nc.tensor.matmul(
                    pvA,
                    lhsT=vaug[:, j * (HL * 65) + hA * 65:
                              j * (HL * 65) + hA * 65 + 65],
                    rhs=pt[:, 0:512],
                    start=(j == 0), stop=(j == JC - 1),
                )
                nc.tensor.matmul(
                    pvB,
                    lhsT=vaug[:, j * (HL * 65) + hB * 65:
                              j * (HL * 65) + hB * 65 + 65],
                    rhs=pt[:, 512:1024],
                    start=(j == 0), stop=(j == JC - 1),
                )

            # software-pipelined by one j-chunk
            emit_sim(0)
            for j in range(JC - 1):
                emit_sim(j + 1)
                emit_pv(j)
                if j % 3 == 1:
                    if mine:
                        mine.popleft()()
                    elif free:
                        free.popleft()()
            emit_pv(JC - 1)

            # normalization off the PE (see module docstring)
            raw = spool.tile([P, 1024], FP, tag="raw", bufs=1)
            nc.vector.tensor_copy(out=raw[0:65, 0:512], in_=pvA)
            nc.vector.tensor_copy(out=raw[0:65, 512:1024], in_=pvB)
            bb = spool.tile([P, 512], FP, tag="bshift", bufs=1)
            nc.sync.dma_start(out=bb[64:128, :], in_=raw[0:64, 512:1024])
            den = spool.tile([1, 1024], FP, tag="den", bufs=1)
            nc.sync.dma_start(out=den, in_=raw[64:65, 0:1024])
            nc.vector.reciprocal_approx_fast(out=den[0:1, 0:512],
                                             in_=den[0:1, 0:512])
            nc.vector.reciprocal_approx_fast(out=den[0:1, 512:1024],
                                             in_=den[0:1, 512:1024])
            bcA = spool.tile([P, 512], FP, tag="bc", bufs=2)
            bcB = spool.tile([P, 512], FP, tag="bc", bufs=2)
            nc.gpsimd.partition_broadcast(bcA, den[0:1, 0:512])
            nc.gpsimd.partition_broadcast(bcB, den[0:1, 512:1024])
            ocol = hp * N + it * 512
            nc.vector.tensor_mul(out=out_s[0:64, ocol:ocol + 512],
                                 in0=raw[0:64, 0:512], in1=bcA[0:64, :])
            nc.vector.tensor_mul(out=out_s[64:128, ocol:ocol + 512],
                                 in0=bb[64:128, :], in1=bcB[64:128, :])

        for hp in range(HPL):
            for it in range(IT):
                attention_pass(
                    hp, it,
                    emit_v_inline=(hp == 0 and it == 0),
                    mine=pinned[hp * IT + it],
                )
                if hp == 1:
                    # this i-tile's out_s is now complete for both inner
                    # chunks: queue its y tiles
                    for oc in range(CC):
                        free.append(lambda oc=oc, nt2=it: emit_y(oc, nt2))
        while free:
            free.popleft()()


# ------------------------- host-side shard / gather -------------------------

def _shard_inputs(x, context, Wq, Wk, Wv, Wo, bo):
    """Build the per-core DRAM images (all [128, free])."""
    def chunk_rows(a):
        n = a.shape[1]
        return np.ascontiguousarray(
            a.reshape(-1, P, n).transpose(1, 0, 2).reshape(P, -1))

    WqT, WkT, WvT, WoT = Wq.T, Wk.T, Wv.T, Wo.T
    zeros_bo = np.zeros((P, CC), np.float32)

    in_maps = []
    for c in range(N_CORES):
        b, hg = c // 2, c % 2
        cols = slice(hg * CIN, (hg + 1) * CIN)
        x_s = x[b].reshape(CC, P, IT, 512).transpose(1, 2, 0, 3).reshape(P, IT * CC * 512)
        ctx_s = context[b].reshape(CC, P, NT, 512).transpose(1, 2, 0, 3).reshape(P, NT * CC * 512)
        in_maps.append({
            "x": np.ascontiguousarray(x_s.astype(NP_BF16)),
            "ctx": np.ascontiguousarray(ctx_s.astype(NP_BF16)),
            "wq": chunk_rows(np.ascontiguousarray(WqT[:, cols])).astype(NP_BF16),
            "wk": chunk_rows(np.ascontiguousarray(WkT[:, cols])).astype(NP_BF16),
            "wv": chunk_rows(np.ascontiguousarray(WvT[:, cols])).astype(NP_BF16),
            "wo": chunk_rows(
                np.ascontiguousarray(WoT[hg * CIN:(hg + 1) * CIN, :])).astype(NP_BF16),
            "bo": np.ascontiguousarray(bo.reshape(CC, P).T) if hg == 0 else zeros_bo,
        })
    return in_maps


def _gather_outputs(results):
    y_full = np.empty((4, C, N), np.float32)
    for b in range(4):
        acc = None
        for hg in range(2):
            y_s = results[2 * b + hg]["y"].astype(np.float32)  # [128, 4*2048]
            part = y_s.reshape(P, CC, N).transpose(1, 0, 2).reshape(C, N)
            acc = part if acc is None else acc + part
        y_full[b] = acc
    return y_full


_PROGRAM = None


def _get_program():
    global _PROGRAM
    if _PROGRAM is None:
        _PROGRAM = _build_program()
    return _PROGRAM


def run(trace=False, **inputs):
    nc = _get_program()
    in_maps = _shard_inputs(
        np.asarray(inputs["x"], np.float32),
        np.asarray(inputs["context"], np.float32),
        np.asarray(inputs["Wq"], np.float32),
        np.asarray(inputs["Wk"], np.float32),
        np.asarray(inputs["Wv"], np.float32),
        np.asarray(inputs["Wo"], np.float32),
        np.asarray(inputs["bo"], np.float32),
    )
    res = run_bass_kernel_spmd(nc, in_maps, list(range(N_CORES)), trace=trace)
    return _gather_outputs(res.results), res


def kernel(**inputs):
    out, _ = run(trace=False, **inputs)
    return out


# revision 10
# speedup vs baseline: 1.0102x; 1.0102x over previous
"""Trainium2 Bass kernel for nn_CrossAttention (B=4, H=8, D=64, C=512, N=M=2048).

Sharding: 8 cores = batch (4) x head-group (2). Core c handles batch b=c//2
and heads hg*4..hg*4+4 with hg=c%2 (tensor parallel on inner_dim). Each core
emits a full-shape partial y (its Wo column block times its heads' attention
output, plus bias on hg==0); the host unshard sums the two partials per batch.

Per-core math (all on-device):
  q  = Wq[hg] @ x             [256, 2048]
  k  = Wk[hg] @ ctx           [256, 2048]
  vT = ctx.T @ Wv[hg].T       [2048, 256]   (built into the ones-augmented
                                             per-j-chunk layout for PV)
  per local head h: simT[j,i] = sum_d k[d,j] q[d,i];  p = exp(simT/8)
  out_aug = [vT_h | 1].T @ p  [65, 2048]    (row 64 = softmax denominator)
  out_h   = out_aug[:64] / out_aug[64]
  y_part = Wo[:, hg cols] @ out (+ bo)      [512, 2048]

All matmul operands are bf16 (fp32 PSUM accumulation): LDWEIGHTS gets the
fast (FWL) path and input DMA halves. The two sim matmuls per j-chunk use
K=64 row groups at base partitions 0/64 and are emitted back-to-back so they
run concurrently in disjoint PE row groups. Softmax skips max-subtraction
(|sim/8| small for this distribution); the denominator rides the PV matmul
as a ones column. Division happens off the PE: DVE drain, approx reciprocal,
gpsimd partition-broadcast, multiply. The y projection accumulates both
inner-chunk matmuls in PSUM (one DVE bias-add readout per tile) and is
drained pass-by-pass so only the last i-tile's y work trails the final pass.
"""

from collections import deque
from contextlib import ExitStack

import numpy as np
import ml_dtypes

import concourse.bass as bass
import concourse.mybir as mybir
import concourse.tile as tile
from concourse import bacc
from concourse.bass_utils import run_bass_kernel_spmd

FP = mybir.dt.float32
BF16 = mybir.dt.bfloat16
EXP = mybir.ActivationFunctionType.Exp
NP_BF16 = ml_dtypes.bfloat16

P = 128
H, D = 8, 64
C = 512             # query_dim == full inner_dim
N, M = 2048, 2048
HL = 4              # local heads per core
HPL = 2             # local head pairs
CIN = HL * D        # local inner dim = 256
CC = C // P         # 4 contraction chunks for q/k/v projections
IT = N // 512       # 4 query i-tiles
JC = M // P         # 16 context chunks
NT = M // 512       # 4 context column blocks
ICY = CIN // P      # 2 inner chunks for the y projection
SCALE = float(D) ** -0.5
N_CORES = 8
NWARM = 24


def _build_program():
    nc = bacc.Bacc("TRN2")
    x = nc.dram_tensor("x", [P, IT * CC * 512], BF16, kind="ExternalInput")
    ctx = nc.dram_tensor("ctx", [P, NT * CC * 512], BF16, kind="ExternalInput")
    wq = nc.dram_tensor("wq", [P, CC * CIN], BF16, kind="ExternalInput")
    wk = nc.dram_tensor("wk", [P, CC * CIN], BF16, kind="ExternalInput")
    wv = nc.dram_tensor("wv", [P, CC * CIN], BF16, kind="ExternalInput")
    wo = nc.dram_tensor("wo", [P, ICY * 512], BF16, kind="ExternalInput")
    bo = nc.dram_tensor("bo", [P, CC], FP, kind="ExternalInput")
    y = nc.dram_tensor("y", [P, CC * N], BF16, kind="ExternalOutput")

    with tile.TileContext(nc) as tc:
        _emit(tc, x, ctx, wq, wk, wv, wo, bo, y)
    nc.finalize()
    return nc


def _emit(tc, x, ctx, wq, wk, wv, wo, bo, y):
    nc = tc.nc
    with ExitStack() as st:
        wpool = st.enter_context(tc.tile_pool(name="weights", bufs=1))
        apool = st.enter_context(tc.tile_pool(name="acts", bufs=1))
        ppool = st.enter_context(tc.tile_pool(name="pexp", bufs=3))
        spool = st.enter_context(tc.tile_pool(name="small", bufs=2))
        psim = st.enter_context(tc.tile_pool(name="psim", bufs=2, space="PSUM"))
        ppv = st.enter_context(tc.tile_pool(name="ppv", bufs=2, space="PSUM"))
        pmisc = st.enter_context(tc.tile_pool(name="pmisc", bufs=2, space="PSUM"))

        # ---- input loads, ordered so pass(0,0) starts ASAP ----
        # pass(0,0) j=0 needs only wk+ctx0 (k chunk 0), wv (inline v), x0+wq
        # (q tile 0); later ctx/x blocks stream in under the pass.
        wq_s = wpool.tile([P, CC * CIN], BF16, tag="wq")
        nc.sync.dma_start(out=wq_s, in_=wq[:, :])
        x_s = apool.tile([P, IT * CC * 512], BF16, tag="x")
        nc.sync.dma_start(out=x_s[:, 0:2048], in_=x[:, 0:2048])
        wk_s = wpool.tile([P, CC * CIN], BF16, tag="wk")
        nc.sync.dma_start(out=wk_s, in_=wk[:, :])
        ctx_s = apool.tile([P, NT * CC * 512], BF16, tag="ctx")
        nc.sync.dma_start(out=ctx_s[:, 0:2048], in_=ctx[:, 0:2048])
        wv_s = wpool.tile([P, CC * CIN], BF16, tag="wv")
        nc.sync.dma_start(out=wv_s, in_=wv[:, :])
        for nb in range(1, NT):
            nc.sync.dma_start(
                out=ctx_s[:, nb * 2048:(nb + 1) * 2048],
                in_=ctx[:, nb * 2048:(nb + 1) * 2048],
            )
        for it in range(1, IT):
            nc.sync.dma_start(
                out=x_s[:, it * 2048:(it + 1) * 2048],
                in_=x[:, it * 2048:(it + 1) * 2048],
            )
        wo_s = wpool.tile([P, ICY * 512], BF16, tag="wo")
        nc.sync.dma_start(out=wo_s, in_=wo[:, :])
        bo_s = wpool.tile([P, CC], FP, tag="bo")
        nc.sync.dma_start(out=bo_s, in_=bo[:, :])

        # ---- persistent SBUF intermediates ----
        # q/k: local head pair hp at cols hp*2048 + it(or nt)*512 + n
        q_s = apool.tile([P, HPL * N], BF16, tag="q")
        k_s = apool.tile([P, HPL * M], BF16, tag="k")
        # v aug: j-chunk j at cols j*(HL*65), local head h at sub-cols h*65
        vaug = apool.tile([P, JC * (HL * 65)], BF16, tag="vaug")
        # attention out, local inner chunk ic at cols ic*2048 + it*512
        out_s = apool.tile([P, ICY * N], BF16, tag="out")
        # full-shape partial y staging (DMA'd out per tile)
        y_acc = apool.tile([P, CC * N], BF16, tag="yacc")
        # fp32 ones staging for vaug ones columns
        ones_s = wpool.tile([P, P], FP, tag="ones")
        nc.vector.memset(ones_s, 1.0)
        vaug4 = vaug.rearrange("p (j h e) -> p j h e", j=JC, h=HL)
        ones4 = ones_s[:, 0:JC * HL].rearrange("p (j h e) -> p j h e", j=JC, h=HL)
        nc.vector.tensor_copy(out=vaug4[:, :, :, 64:65], in_=ones4)

        # HAM warmup: burn matmuls on the ones tile during the initial DMA
        # wait so the first projections run at the full 2.4GHz clock.
        warm = pmisc.tile([P, 512], FP, tag="scratch", name="warm")
        for w in range(NWARM):
            nc.tensor.matmul(warm[:, 0:P], lhsT=ones_s[:, 0:P],
                             rhs=ones_s[:, 0:P],
                             start=(w == 0), stop=(w == NWARM - 1))
        warm_sink = spool.tile([P, P], FP, tag="warmsink", bufs=1)
        nc.vector.tensor_copy(out=warm_sink, in_=warm[:, 0:P])

        def proj_qk(dst, w_s, oc, rhs_of_cc):
            """One [128, 512] q/k projection tile (local head pair oc)."""
            pt = pmisc.tile([P, 512], FP, tag="scratch")
            for cc in range(CC):
                nc.tensor.matmul(
                    pt,
                    lhsT=w_s[:, cc * CIN + oc * P: cc * CIN + (oc + 1) * P],
                    rhs=rhs_of_cc(cc),
                    start=(cc == 0), stop=(cc == CC - 1),
                )
            nc.vector.tensor_copy(out=dst, in_=pt)

        def emit_q(oc, it):
            proj_qk(q_s[:, oc * N + it * 512: oc * N + (it + 1) * 512], wq_s, oc,
                    lambda cc: x_s[:, it * 2048 + cc * 512: it * 2048 + (cc + 1) * 512])

        def emit_k(oc, nt):
            proj_qk(k_s[:, oc * M + nt * 512: oc * M + (nt + 1) * 512], wk_s, oc,
                    lambda cc: ctx_s[:, nt * 2048 + cc * 512: nt * 2048 + (cc + 1) * 512])

        def emit_v(j):
            nb, jm = j // 4, j % 4
            pt = pmisc.tile([P, 512], FP, tag="scratch")
            for cc in range(CC):
                nc.tensor.matmul(
                    pt[:, 0:CIN],
                    lhsT=ctx_s[:, nb * 2048 + cc * 512 + jm * P:
                               nb * 2048 + cc * 512 + (jm + 1) * P],
                    rhs=wv_s[:, cc * CIN:(cc + 1) * CIN],
                    start=(cc == 0), stop=(cc == CC - 1),
                )
            nc.vector.tensor_copy(
                out=vaug4[:, j, :, 0:64],
                in_=pt[:, 0:CIN].rearrange("p (h e) -> p h e", h=HL),
            )

        def emit_y(oc, nt2):
            """Full y output tile (oc, nt2): both inner chunks accumulate in
            PSUM, then one bias-add readout + DMA."""
            pt = pmisc.tile([P, 512], FP, tag="scratch")
            for ic in range(ICY):
                nc.tensor.matmul(
                    pt,
                    lhsT=wo_s[:, ic * 512 + oc * P: ic * 512 + (oc + 1) * P],
                    rhs=out_s[:, ic * N + nt2 * 512: ic * N + (nt2 + 1) * 512],
                    start=(ic == 0), stop=(ic == ICY - 1),
                )
            ysl = y_acc[:, oc * N + nt2 * 512: oc * N + (nt2 + 1) * 512]
            nc.vector.tensor_scalar_add(out=ysl, in0=pt,
                                        scalar1=bo_s[:, oc:oc + 1])
            nc.sync.dma_start(
                out=y[:, oc * N + nt2 * 512: oc * N + (nt2 + 1) * 512],
                in_=ysl)

        # pinned[i]: projection tiles that MUST be emitted during pass i
        # in its drain slots (j = 1, 4, 7, 10, 13), sized so each tile
        # lands before its first consumer; free: y tiles drained
        # opportunistically.
        pinned = {i: deque() for i in range(HPL * IT)}
        # k(0, nt) is consumed from j = 4*nt of pass 0 onward; slots 1/4
        # precede j = 8/12 comfortably (k(0,2) emitted at j=1 finishes ~j=3).
        pinned[0].extend([
            lambda: emit_k(0, 2), lambda: emit_k(0, 3), lambda: emit_q(0, 1)])
        pinned[1].extend([lambda: emit_q(0, 2), lambda: emit_k(1, 0)])
        pinned[2].extend([
            lambda: emit_q(0, 3), lambda: emit_k(1, 1), lambda: emit_k(1, 2)])
        pinned[3].extend([lambda: emit_k(1, 3), lambda: emit_q(1, 0)])
        for it in range(1, IT):
            pinned[3 + it].append(lambda it=it: emit_q(1, it))
        free = deque()

        # upfront: just enough for pass(0,0) j=0..7 — q tile 0, k chunks 0-1
        emit_q(0, 0)
        emit_k(0, 0)
        emit_k(0, 1)

        def attention_pass(hp, it, emit_v_inline, mine):
            hA, hB = 2 * hp, 2 * hp + 1
            pvA = ppv.tile([65, 512], FP, tag="pv")
            pvB = ppv.tile([65, 512], FP, tag="pv")
            qA = q_s[0:64, hp * N + it * 512: hp * N + (it + 1) * 512]
            qB = q_s[64:128, hp * N + it * 512: hp * N + (it + 1) * 512]
            pts = [None] * JC

            def emit_sim(j):
                # Both K=64 halves back-to-back: base partitions 0/64 give
                # disjoint PE row groups, so the two matmuls run concurrently.
                if emit_v_inline:
                    emit_v(j)
                st_t = psim.tile([P, 1024], FP, tag="sim", name="st_t")
                pt = ppool.tile([P, 1024], BF16, tag="p", name="pt")
                pts[j] = (st_t, pt)
                nc.tensor.matmul(
                    st_t[:, 0:512],
                    lhsT=k_s[0:64, hp * M + j * P: hp * M + (j + 1) * P],
                    rhs=qA,
                )
                nc.tensor.matmul(
                    st_t[:, 512:1024],
                    lhsT=k_s[64:128, hp * M + j * P: hp * M + (j + 1) * P],
                    rhs=qB,
                )
                nc.scalar.activation(out=pt, in_=st_t, func=EXP, scale=SCALE)

            def emit_pv(j):
                pt = pts[j][1]
                nc.tensor.matmul(
                    pvA,
                    lhsT=vaug[:, j * (HL * 65) + hA * 65:
                              j * (HL * 65) + hA * 65 + 65],
                    rhs=pt[:, 0:512],
                    start=(j == 0), stop=(j == JC - 1),
                )
                nc.tensor.matmul(
                    pvB,
                    lhsT=vaug[:, j * (HL * 65) + hB * 65:
                              j * (HL * 65) + hB * 65 + 65],
                    rhs=pt[:, 512:1024],
                    start=(j == 0), stop=(j == JC - 1),
                )

            # software-pipelined by one j-chunk
            emit_sim(0)
            for j in range(JC - 1):
                emit_sim(j + 1)
                emit_pv(j)
                if j % 3 == 1:
                    if mine:
                        mine.popleft()()
                    elif free:
                        free.popleft()()
            emit_pv(JC - 1)

            # normalization off the PE (see module docstring)
            raw = spool.tile([P, 1024], FP, tag="raw", bufs=1)
            nc.vector.tensor_copy(out=raw[0:65, 0:512], in_=pvA)
            nc.vector.tensor_copy(out=raw[0:65, 512:1024], in_=pvB)
            bb = spool.tile([P, 512], FP, tag="bshift", bufs=1)
            nc.sync.dma_start(out=bb[64:128, :], in_=raw[0:64, 512:1024])
            den = spool.tile([1, 1024], FP, tag="den", bufs=1)
            nc.sync.dma_start(out=den, in_=raw[64:65, 0:1024])
            nc.vector.reciprocal_approx_fast(out=den[0:1, 0:512],
                                             in_=den[0:1, 0:512])
            nc.vector.reciprocal_approx_fast(out=den[0:1, 512:1024],
                                             in_=den[0:1, 512:1024])
            bcA = spool.tile([P, 512], FP, tag="bc", bufs=2)
            bcB = spool.tile([P, 512], FP, tag="bc", bufs=2)
            nc.gpsimd.partition_broadcast(bcA, den[0:1, 0:512])
            nc.gpsimd.partition_broadcast(bcB, den[0:1, 512:1024])
            ocol = hp * N + it * 512
            nc.vector.tensor_mul(out=out_s[0:64, ocol:ocol + 512],
                                 in0=raw[0:64, 0:512], in1=bcA[0:64, :])
            nc.vector.tensor_mul(out=out_s[64:128, ocol:ocol + 512],
                                 in0=bb[64:128, :], in1=bcB[64:128, :])

        for hp in range(HPL):
            for it in range(IT):
                attention_pass(
                    hp, it,
                    emit_v_inline=(hp == 0 and it == 0),
                    mine=pinned[hp * IT + it],
                )
                if hp == 1:
                    # this i-tile's out_s is now complete for both inner
                    # chunks: queue its y tiles
                    for oc in range(CC):
                        free.append(lambda oc=oc, nt2=it: emit_y(oc, nt2))
        # keep the PE's HAM clock warm through the last pass's normalization
        # chain so the trailing y matmuls run at 2.4GHz, not the cold 1.2.
        warm2 = pmisc.tile([P, 512], FP, tag="scratch", name="warm2")
        for w in range(8):
            nc.tensor.matmul(warm2, lhsT=wq_s[:, 0:P], rhs=x_s[:, 0:512],
                             start=(w == 0), stop=(w == 7))
        warm2_sink = spool.tile([P, P], FP, tag="warmsink2", bufs=1)
        nc.vector.tensor_copy(out=warm2_sink, in_=warm2[:, 0:P])
        while free:
            free.popleft()()


# ------------------------- host-side shard / gather -------------------------

def _shard_inputs(x, context, Wq, Wk, Wv, Wo, bo):
    """Build the per-core DRAM images (all [128, free])."""
    def chunk_rows(a):
        n = a.shape[1]
        return np.ascontiguousarray(
            a.reshape(-1, P, n).transpose(1, 0, 2).reshape(P, -1))

    WqT, WkT, WvT, WoT = Wq.T, Wk.T, Wv.T, Wo.T
    zeros_bo = np.zeros((P, CC), np.float32)

    in_maps = []
    for c in range(N_CORES):
        b, hg = c // 2, c % 2
        cols = slice(hg * CIN, (hg + 1) * CIN)
        x_s = x[b].reshape(CC, P, IT, 512).transpose(1, 2, 0, 3).reshape(P, IT * CC * 512)
        ctx_s = context[b].reshape(CC, P, NT, 512).transpose(1, 2, 0, 3).reshape(P, NT * CC * 512)
        in_maps.append({
            "x": np.ascontiguousarray(x_s.astype(NP_BF16)),
            "ctx": np.ascontiguousarray(ctx_s.astype(NP_BF16)),
            "wq": chunk_rows(np.ascontiguousarray(WqT[:, cols])).astype(NP_BF16),
            "wk": chunk_rows(np.ascontiguousarray(WkT[:, cols])).astype(NP_BF16),
            "wv": chunk_rows(np.ascontiguousarray(WvT[:, cols])).astype(NP_BF16),
            "wo": chunk_rows(
                np.ascontiguousarray(WoT[hg * CIN:(hg + 1) * CIN, :])).astype(NP_BF16),
            "bo": np.ascontiguousarray(bo.reshape(CC, P).T) if hg == 0 else zeros_bo,
        })
    return in_maps


def _gather_outputs(results):
    y_full = np.empty((4, C, N), np.float32)
    for b in range(4):
        acc = None
        for hg in range(2):
            y_s = results[2 * b + hg]["y"].astype(np.float32)  # [128, 4*2048]
            part = y_s.reshape(P, CC, N).transpose(1, 0, 2).reshape(C, N)
            acc = part if acc is None else acc + part
        y_full[b] = acc
    return y_full


_PROGRAM = None


def _get_program():
    global _PROGRAM
    if _PROGRAM is None:
        _PROGRAM = _build_program()
    return _PROGRAM


def run(trace=False, **inputs):
    nc = _get_program()
    in_maps = _shard_inputs(
        np.asarray(inputs["x"], np.float32),
        np.asarray(inputs["context"], np.float32),
        np.asarray(inputs["Wq"], np.float32),
        np.asarray(inputs["Wk"], np.float32),
        np.asarray(inputs["Wv"], np.float32),
        np.asarray(inputs["Wo"], np.float32),
        np.asarray(inputs["bo"], np.float32),
    )
    res = run_bass_kernel_spmd(nc, in_maps, list(range(N_CORES)), trace=trace)
    return _gather_outputs(res.results), res


def kernel(**inputs):
    out, _ = run(trace=False, **inputs)
    return out
